# revision 6
# baseline (speedup 1.0000x reference)
"""LSTM-cell critic (nn_LCritic) Trainium2 Bass kernel.

Strategy: pure data-parallel over 8 NeuronCores (batch 32768 -> 4096/core).
Host-side prep transposes the activations so the contraction dim (features)
lands on SBUF partitions, letting every GEMM run transpose-free on-device:

  gatesT[256, b] = Wi^T @ xT + Wh^T @ hT   (PE, fp16 in / fp32 psum)
  i,f,g,o are partition slices of psum  -> ACT sigmoid/tanh with fused bias
  c' = f*c + i*g ; h' = o*tanh(c')         (DVE, fp32)
  out = tanh(Wo^T @ h' + bo)               (PE fp32 + ACT)

Outputs come back feature-major; the host transposes them back.
"""

import sys

sys.path.insert(0, "/opt/trn_rl_repo")

import numpy as np

import concourse.tile as tile
from concourse import bacc, mybir
from concourse.bass_utils import run_bass_kernel_spmd

B = 32768
NCORES = 8
BL = B // NCORES  # 4096 batch rows per core
S_IN = 2048  # state features
A_IN = 512  # action features
K_IN = S_IN + A_IN  # 2560
H = 64
G = 4 * H  # 256 fused gate dim, order [i, f, g, o]
NB = 512  # batch columns per chunk
NCHUNK = BL // NB  # 8

F16 = mybir.dt.float16
F32 = mybir.dt.float32

_NC_CACHE = {}


def _build():
    nc = bacc.Bacc("TRN2", target_bir_lowering=False, debug=False)

    # x packed chunk-tiled: xp[j, p, kk, n] = x[j*NB+n, kk*128+p]
    # -> per chunk one DMA, 20KB contiguous per partition
    xp_d = nc.dram_tensor("xp", [NCHUNK, 128, K_IN // 128, NB], F16, kind="ExternalInput")
    hT_d = nc.dram_tensor("hT", [H, BL], F16, kind="ExternalInput")
    cT_d = nc.dram_tensor("cT", [H, BL], F32, kind="ExternalInput")
    Wi_d = nc.dram_tensor("Wi", [K_IN, G], F16, kind="ExternalInput")
    Wh_d = nc.dram_tensor("Wh", [H, G], F16, kind="ExternalInput")
    Wo_d = nc.dram_tensor("Wo", [H, 1], F32, kind="ExternalInput")
    bias_d = nc.dram_tensor("bias", [G], F32, kind="ExternalInput")
    bo_d = nc.dram_tensor("bo", [1, 1], F32, kind="ExternalInput")

    hTo_d = nc.dram_tensor("hT_out", [H, BL], F32, kind="ExternalOutput")
    cTo_d = nc.dram_tensor("cT_out", [H, BL], F32, kind="ExternalOutput")
    outT_d = nc.dram_tensor("outT", [1, BL], F32, kind="ExternalOutput")

    # k-chunked weight view: k = o*128 + p with p on partitions
    Wi_r = Wi_d.ap().rearrange("(o p) m -> p o m", p=128)  # [128, 20, G]
    bias_r = bias_d.ap().rearrange("(o p) -> p o", p=128)  # [128, 2]

    KC = K_IN // 128  # 20 k-chunks

    with tile.TileContext(nc) as tc:
        with (
            tc.tile_pool(name="const", bufs=1) as cpool,
            tc.tile_pool(name="xin", bufs=3) as xpool,
            tc.tile_pool(name="acts", bufs=3) as spool,
            tc.tile_pool(name="elem", bufs=3) as epool,
            tc.tile_pool(name="psum", bufs=2, space="PSUM") as pspool,
            tc.tile_pool(name="psumo", bufs=2, space="PSUM") as pspool_o,
        ):
            # --- persistent weights / state ---
            Wi_sb = cpool.tile([128, KC, G], F16)
            nc.sync.dma_start(Wi_sb[:], Wi_r)
            Wh_sb = cpool.tile([H, G], F16)
            nc.sync.dma_start(Wh_sb[:], Wh_d.ap())
            Wo_sb = cpool.tile([H, 1], F32)
            nc.sync.dma_start(Wo_sb[:], Wo_d.ap())
            bias_sb = cpool.tile([128, 2], F32)
            nc.sync.dma_start(bias_sb[:], bias_r)
            bo_sb = cpool.tile([1, 1], F32)
            nc.sync.dma_start(bo_sb[:], bo_d.ap())
            hT_sb = cpool.tile([H, BL], F16)
            nc.sync.dma_start(hT_sb[:], hT_d.ap())
            cT_sb = cpool.tile([H, BL], F32)
            nc.sync.dma_start(cT_sb[:], cT_d.ap())

            for j in range(NCHUNK):
                jsl = slice(j * NB, (j + 1) * NB)

                # --- load xT chunk: one contiguous 2.6MB DMA ---
                xt = xpool.tile([128, KC, NB], F16, tag="x")
                nc.sync.dma_start(xt[:], xp_d.ap()[j])

                # --- fused-gate GEMM: psum0 = gates[0:128] (i|f), psum1 = (g|o) ---
                ps0 = pspool.tile([128, NB], F32, tag="ps0")
                ps1 = pspool.tile([128, NB], F32, tag="ps1")
                for kk in range(KC):
                    rhs = xt[:, kk, :]
                    st = kk == 0
                    nc.tensor.matmul(
                        ps0[:], Wi_sb[:, kk, 0:128], rhs, start=st, stop=False
                    )
                    nc.tensor.matmul(
                        ps1[:], Wi_sb[:, kk, 128:256], rhs, start=st, stop=False
                    )
                hrhs = hT_sb[:, jsl]
                nc.tensor.matmul(ps0[:], Wh_sb[:, 0:128], hrhs, start=False, stop=True)
                nc.tensor.matmul(ps1[:], Wh_sb[:, 128:256], hrhs, start=False, stop=True)

                # --- activations (bias fused): S[:,0]=sig(i|f), S[:,1]=[tanh g; sig o]
                S = spool.tile([128, 2, NB], F32, tag="S")
                nc.scalar.activation(
                    S[:, 0, :],
                    ps0[:],
                    mybir.ActivationFunctionType.Sigmoid,
                    bias=bias_sb[:, 0:1],
                )
                nc.scalar.activation(
                    S[0:H, 1, :],
                    ps1[0:H, :],
                    mybir.ActivationFunctionType.Tanh,
                    bias=bias_sb[0:H, 1:2],
                )
                nc.scalar.activation(
                    S[H:128, 1, :],
                    ps1[H:128, :],
                    mybir.ActivationFunctionType.Sigmoid,
                    bias=bias_sb[H:128, 1:2],
                )

                # --- shift f (and o) down to partitions 0:64 ---
                fo = epool.tile([H, 2, NB], F32, tag="fo")
                nc.gpsimd.dma_start(fo[:], S[H:128, :, :])

                # --- elementwise LSTM cell (partitions 0:64, fp32) ---
                ig = epool.tile([H, NB], F32, tag="ig")
                nc.vector.tensor_mul(ig[:], S[0:H, 0, :], S[0:H, 1, :])  # i*g
                ct = epool.tile([H, NB], F32, tag="ct")
                nc.vector.tensor_mul(ct[:], fo[:, 0, :], cT_sb[:, jsl])  # f*c
                nc.vector.tensor_add(ct[:], ct[:], ig[:])  # c'
                nc.gpsimd.dma_start(cTo_d.ap()[:, jsl], ct[:])

                tc_t = epool.tile([H, NB], F32, tag="tanhc")
                nc.scalar.activation(
                    tc_t[:], ct[:], mybir.ActivationFunctionType.Tanh
                )
                ht = epool.tile([H, NB], F32, tag="ht")
                nc.vector.tensor_mul(ht[:], fo[:, 1, :], tc_t[:])  # h'
                nc.gpsimd.dma_start(hTo_d.ap()[:, jsl], ht[:])

                # --- value head: out = tanh(Wo^T @ h' + bo) ---
                pso = pspool_o.tile([1, NB], F32, tag="pso")
                nc.tensor.matmul(pso[:], Wo_sb[:], ht[:], start=True, stop=True)
                ot = epool.tile([1, NB], F32, tag="ot")
                nc.scalar.activation(
                    ot[:],
                    pso[:],
                    mybir.ActivationFunctionType.Tanh,
                    bias=bo_sb[0:1, 0:1],
                )
                nc.gpsimd.dma_start(outT_d.ap()[:, jsl], ot[:])

    nc.compile()
    return nc


def _get_nc():
    if "nc" not in _NC_CACHE:
        _NC_CACHE["nc"] = _build()
    return _NC_CACHE["nc"]


def _prep_in_maps(state, action, hidden_state, cell_state, Wi, bi, Wh, bh, Wo, bo):
    Wi16 = np.ascontiguousarray(Wi.astype(np.float16))
    Wh16 = np.ascontiguousarray(Wh.astype(np.float16))
    Wo32 = np.ascontiguousarray(Wo.astype(np.float32))
    bias = (bi.astype(np.float64) + bh.astype(np.float64)).astype(np.float32)
    bo32 = np.asarray(bo, dtype=np.float32).reshape(1, 1)

    KC = K_IN // 128
    KS = S_IN // 128
    in_maps = []
    for c in range(NCORES):
        sl = slice(c * BL, (c + 1) * BL)
        # xp[j, p, kk, n] = x[j*NB+n, kk*128+p]
        xp = np.empty((NCHUNK, 128, KC, NB), np.float16)
        xp[:, :, :KS, :] = (
            state[sl].reshape(NCHUNK, NB, KS, 128).transpose(0, 3, 2, 1)
        )
        xp[:, :, KS:, :] = (
            action[sl].reshape(NCHUNK, NB, KC - KS, 128).transpose(0, 3, 2, 1)
        )
        in_maps.append(
            {
                "xp": xp,
                "hT": np.ascontiguousarray(hidden_state[sl].T.astype(np.float16)),
                "cT": np.ascontiguousarray(cell_state[sl].T.astype(np.float32)),
                "Wi": Wi16,
                "Wh": Wh16,
                "Wo": Wo32,
                "bias": bias,
                "bo": bo32,
            }
        )
    return in_maps


def run(inputs, trace=False):
    """Returns ((out, h_t, c_t), BassKernelResults)."""
    nc = _get_nc()
    in_maps = _prep_in_maps(**inputs)
    res = run_bass_kernel_spmd(nc, in_maps, list(range(NCORES)), trace=trace)

    out = np.empty((B, 1), np.float32)
    h_t = np.empty((B, H), np.float32)
    c_t = np.empty((B, H), np.float32)
    for c in range(NCORES):
        sl = slice(c * BL, (c + 1) * BL)
        r = res.results[c]
        h_t[sl] = np.asarray(r["hT_out"]).T
        c_t[sl] = np.asarray(r["cT_out"]).T
        out[sl, 0] = np.asarray(r["outT"])[0]
    return (out, h_t, c_t), res


def kernel(**inputs):
    return run(inputs)[0]


# revision 9
# speedup vs baseline: 1.1756x; 1.1756x over previous
"""LSTM-cell critic (nn_LCritic) Trainium2 Bass kernel.

Strategy: pure data-parallel over 8 NeuronCores (batch 32768 -> 4096/core).
Host-side prep transposes the activations so the contraction dim (features)
lands on SBUF partitions, letting every GEMM run transpose-free on-device:

  gatesT[256, b] = Wi^T @ xT + Wh^T @ hT   (PE, fp16 in / fp32 psum)
  i,f,g,o are partition slices of psum  -> ACT sigmoid/tanh with fused bias
  c' = f*c + i*g ; h' = o*tanh(c')         (DVE, fp32)
  out = tanh(Wo^T @ h' + bo)               (PE fp32 + ACT)

Outputs come back feature-major; the host transposes them back.
"""

import sys

sys.path.insert(0, "/opt/trn_rl_repo")

import numpy as np

import concourse.tile as tile
from concourse import bacc, mybir
from concourse.bass_utils import run_bass_kernel_spmd

B = 32768
NCORES = 8
BL = B // NCORES  # 4096 batch rows per core
S_IN = 2048  # state features
A_IN = 512  # action features
K_IN = S_IN + A_IN  # 2560
H = 64
G = 4 * H  # 256 fused gate dim, order [i, f, g, o]
NB = 512  # batch columns per chunk
NCHUNK = BL // NB  # 8

F16 = mybir.dt.float16
F32 = mybir.dt.float32

_NC_CACHE = {}


def _build():
    nc = bacc.Bacc("TRN2", target_bir_lowering=False, debug=False)

    # x packed chunk-tiled: xp[j, p, kk, n] = x[j*NB+n, kk*128+p]
    # -> per chunk one DMA, 20KB contiguous per partition
    xp_d = nc.dram_tensor("xp", [NCHUNK, 128, K_IN // 128, NB], F16, kind="ExternalInput")
    hT_d = nc.dram_tensor("hT", [H, BL], F16, kind="ExternalInput")
    cT_d = nc.dram_tensor("cT", [H, BL], F32, kind="ExternalInput")
    Wi_d = nc.dram_tensor("Wi", [K_IN, G], F16, kind="ExternalInput")
    Wh_d = nc.dram_tensor("Wh", [H, G], F16, kind="ExternalInput")
    Wo_d = nc.dram_tensor("Wo", [H, 1], F32, kind="ExternalInput")
    bias_d = nc.dram_tensor("bias", [G], F32, kind="ExternalInput")
    bo_d = nc.dram_tensor("bo", [1, 1], F32, kind="ExternalInput")

    hTo_d = nc.dram_tensor("hT_out", [H, BL], F32, kind="ExternalOutput")
    cTo_d = nc.dram_tensor("cT_out", [H, BL], F32, kind="ExternalOutput")
    outT_d = nc.dram_tensor("outT", [1, BL], F32, kind="ExternalOutput")

    # k-chunked weight view: k = o*128 + p with p on partitions
    Wi_r = Wi_d.ap().rearrange("(o p) m -> p o m", p=128)  # [128, 20, G]
    bias_r = bias_d.ap().rearrange("(o p) -> p o", p=128)  # [128, 2]

    KC = K_IN // 128  # 20 k-chunks

    with tile.TileContext(nc) as tc:
        with (
            tc.tile_pool(name="const", bufs=1) as cpool,
            tc.tile_pool(name="xin", bufs=3) as xpool,
            tc.tile_pool(name="acts", bufs=5) as spool,
            tc.tile_pool(name="elem", bufs=6) as epool,
            tc.tile_pool(name="psum", bufs=3, space="PSUM") as pspool,
            tc.tile_pool(name="psumo", bufs=2, space="PSUM") as pspool_o,
        ):
            # --- persistent weights / state ---
            Wi_sb = cpool.tile([128, KC, G], F16)
            nc.sync.dma_start(Wi_sb[:], Wi_r)
            Wh_sb = cpool.tile([H, G], F16)
            nc.sync.dma_start(Wh_sb[:], Wh_d.ap())
            Wo_sb = cpool.tile([H, 1], F32)
            nc.sync.dma_start(Wo_sb[:], Wo_d.ap())
            bias_sb = cpool.tile([128, 2], F32)
            nc.sync.dma_start(bias_sb[:], bias_r)
            bo_sb = cpool.tile([1, 1], F32)
            nc.sync.dma_start(bo_sb[:], bo_d.ap())
            hT_sb = cpool.tile([H, BL], F16)
            nc.sync.dma_start(hT_sb[:], hT_d.ap())
            cT_sb = cpool.tile([H, BL], F32)
            nc.sync.dma_start(cT_sb[:], cT_d.ap())

            for j in range(NCHUNK):
                jsl = slice(j * NB, (j + 1) * NB)

                # --- load xT chunk: two contiguous 1.3MB DMAs (finer deps) ---
                KH = KC // 2
                xlo = xpool.tile([128, KH, NB], F16, tag="xlo")
                nc.sync.dma_start(xlo[:], xp_d.ap()[j, :, :KH, :])
                xhi = xpool.tile([128, KC - KH, NB], F16, tag="xhi")
                nc.sync.dma_start(xhi[:], xp_d.ap()[j, :, KH:, :])

                # --- fused-gate GEMM: psum0 = gates[0:128] (i|f), psum1 = (g|o) ---
                ps0 = pspool.tile([128, NB], F32, tag="ps0")
                ps1 = pspool.tile([128, NB], F32, tag="ps1")
                for kk in range(KC):
                    rhs = xlo[:, kk, :] if kk < KH else xhi[:, kk - KH, :]
                    st = kk == 0
                    nc.tensor.matmul(
                        ps0[:], Wi_sb[:, kk, 0:128], rhs, start=st, stop=False
                    )
                    nc.tensor.matmul(
                        ps1[:], Wi_sb[:, kk, 128:256], rhs, start=st, stop=False
                    )
                hrhs = hT_sb[:, jsl]
                nc.tensor.matmul(ps0[:], Wh_sb[:, 0:128], hrhs, start=False, stop=True)
                nc.tensor.matmul(ps1[:], Wh_sb[:, 128:256], hrhs, start=False, stop=True)

                # --- activations (bias fused): S[:,0]=sig(i|f), S[:,1]=[tanh g; sig o]
                S = spool.tile([128, 2, NB], F32, tag="S")
                nc.scalar.activation(
                    S[:, 0, :],
                    ps0[:],
                    mybir.ActivationFunctionType.Sigmoid,
                    bias=bias_sb[:, 0:1],
                )
                nc.scalar.activation(
                    S[0:H, 1, :],
                    ps1[0:H, :],
                    mybir.ActivationFunctionType.Tanh,
                    bias=bias_sb[0:H, 1:2],
                )
                nc.scalar.activation(
                    S[H:128, 1, :],
                    ps1[H:128, :],
                    mybir.ActivationFunctionType.Sigmoid,
                    bias=bias_sb[H:128, 1:2],
                )

                # --- shift f (and o) down to partitions 0:64 ---
                fo = epool.tile([H, 2, NB], F32, tag="fo")
                nc.sync.dma_start(fo[:], S[H:128, :, :])

                # --- elementwise LSTM cell (partitions 0:64, fp32) ---
                ig = epool.tile([H, NB], F32, tag="ig")
                nc.vector.tensor_mul(ig[:], S[0:H, 0, :], S[0:H, 1, :])  # i*g
                ct = epool.tile([H, NB], F32, tag="ct")
                nc.vector.tensor_mul(ct[:], fo[:, 0, :], cT_sb[:, jsl])  # f*c
                nc.vector.tensor_add(ct[:], ct[:], ig[:])  # c'
                nc.gpsimd.dma_start(cTo_d.ap()[:, jsl], ct[:])

                tc_t = epool.tile([H, NB], F32, tag="tanhc")
                nc.scalar.activation(
                    tc_t[:], ct[:], mybir.ActivationFunctionType.Tanh
                )
                ht = epool.tile([H, NB], F32, tag="ht")
                nc.vector.tensor_mul(ht[:], fo[:, 1, :], tc_t[:])  # h'
                nc.gpsimd.dma_start(hTo_d.ap()[:, jsl], ht[:])

                # --- value head: out = tanh(Wo^T @ h' + bo) ---
                pso = pspool_o.tile([1, NB], F32, tag="pso")
                nc.tensor.matmul(pso[:], Wo_sb[:], ht[:], start=True, stop=True)
                ot = epool.tile([1, NB], F32, tag="ot")
                nc.scalar.activation(
                    ot[:],
                    pso[:],
                    mybir.ActivationFunctionType.Tanh,
                    bias=bo_sb[0:1, 0:1],
                )
                nc.gpsimd.dma_start(outT_d.ap()[:, jsl], ot[:])

    nc.compile()
    return nc


def _get_nc():
    if "nc" not in _NC_CACHE:
        _NC_CACHE["nc"] = _build()
    return _NC_CACHE["nc"]


def _prep_in_maps(state, action, hidden_state, cell_state, Wi, bi, Wh, bh, Wo, bo):
    Wi16 = np.ascontiguousarray(Wi.astype(np.float16))
    Wh16 = np.ascontiguousarray(Wh.astype(np.float16))
    Wo32 = np.ascontiguousarray(Wo.astype(np.float32))
    bias = (bi.astype(np.float64) + bh.astype(np.float64)).astype(np.float32)
    bo32 = np.asarray(bo, dtype=np.float32).reshape(1, 1)

    KC = K_IN // 128
    KS = S_IN // 128
    in_maps = []
    for c in range(NCORES):
        sl = slice(c * BL, (c + 1) * BL)
        # xp[j, p, kk, n] = x[j*NB+n, kk*128+p]
        xp = np.empty((NCHUNK, 128, KC, NB), np.float16)
        xp[:, :, :KS, :] = (
            state[sl].reshape(NCHUNK, NB, KS, 128).transpose(0, 3, 2, 1)
        )
        xp[:, :, KS:, :] = (
            action[sl].reshape(NCHUNK, NB, KC - KS, 128).transpose(0, 3, 2, 1)
        )
        in_maps.append(
            {
                "xp": xp,
                "hT": np.ascontiguousarray(hidden_state[sl].T.astype(np.float16)),
                "cT": np.ascontiguousarray(cell_state[sl].T.astype(np.float32)),
                "Wi": Wi16,
                "Wh": Wh16,
                "Wo": Wo32,
                "bias": bias,
                "bo": bo32,
            }
        )
    return in_maps


def run(inputs, trace=False):
    """Returns ((out, h_t, c_t), BassKernelResults)."""
    nc = _get_nc()
    in_maps = _prep_in_maps(**inputs)
    res = run_bass_kernel_spmd(nc, in_maps, list(range(NCORES)), trace=trace)

    out = np.empty((B, 1), np.float32)
    h_t = np.empty((B, H), np.float32)
    c_t = np.empty((B, H), np.float32)
    for c in range(NCORES):
        sl = slice(c * BL, (c + 1) * BL)
        r = res.results[c]
        h_t[sl] = np.asarray(r["hT_out"]).T
        c_t[sl] = np.asarray(r["cT_out"]).T
        out[sl, 0] = np.asarray(r["outT"])[0]
    return (out, h_t, c_t), res


def kernel(**inputs):
    return run(inputs)[0]
